# revision 2
# baseline (speedup 1.0000x reference)
"""MetaAttention Trainium2 kernel.

Problem: B=2, T=2048, C=2048, H=16 heads (D=128), 1024-token memory bank
appended (unprojected) to K/V, causal mask over real tokens, memory tokens
attendable by every query; QKV projection + output projection.

Sharding: 8 cores = 2 batch groups x 4 head groups (4 heads each).
Per core: QKV projection for its 4 heads' columns over its batch,
attention for 4 (head) x 1 (batch), partial output projection with its
head-rows of W_proj. Host sums the 4 partials per batch.

All matmuls run in float32r (full-rate fp32 path, ~1e-4 accurate).
Layouts are chosen so no on-device transposes are needed anywhere:
  scoresT[s,t] = KT_chunk.T @ qT      (lhsT = KT chunk [d,s])
  OT[d,t]     += V_chunk.T @ expT     (lhsT = V chunk [s,d], rhs = expT[s,t])
  y[t,n]      += OT_chunk.T @ Wp      (lhsT = OT chunk [d,t])
Softmax denominator: DVE-accumulate expT chunks, then a ones-matmul
reduces partitions; reciprocal broadcast back via a second ones-matmul.
Softmax skips max-subtraction (scores are O(5), exp is safe in fp32).
"""

import contextlib
import os
import sys
import types

import numpy as np

for _p in ("/opt/trn_rl_repo", "/root/.axon_site/_ro/trn_rl_repo"):
    if os.path.isdir(_p) and _p not in sys.path:
        sys.path.append(_p)

import concourse.bass as bass  # noqa: E402
import concourse.mybir as mybir  # noqa: E402
import concourse.tile as tile  # noqa: E402
from concourse import bacc  # noqa: E402
from concourse.bass_utils import run_bass_kernel_spmd  # noqa: E402

F32 = mybir.dt.float32
F32R = mybir.dt.float32r
EXP = mybir.ActivationFunctionType.Exp

B, T, C = 2, 2048, 2048
H, D, MEM = 16, 128, 1024
HPG = 4            # heads per group (per core)
HG = H // HPG      # head groups
CORES = 8
TB = 512           # t-block width
NTB = T // TB      # 4 t-blocks
NCH = C // 128     # 16 contraction chunks
NMC = MEM // 128   # 8 memory s-chunks
SCALE = 1.0 / float(np.sqrt(D))
NEGMASK = -1.0e32  # pre-scale mask addend; exp(SCALE*x + SCALE*NEGMASK) == 0

LAST_EXEC_TIME_NS = None
_CACHE = {}


def _install_ntff_hook():
    """Register the axon NTFF profiling hook if the image lacks antenv.axon_hooks."""
    try:
        import antenv  # noqa: PLC0415

        if "antenv.axon_hooks" in sys.modules:
            return
        mod = types.ModuleType("antenv.axon_hooks")
        state = {"hook": None}
        mod.set_axon_ntff_profile_hook = lambda h: state.__setitem__("hook", h)
        mod.get_axon_ntff_profile_hook = lambda: state["hook"]
        sys.modules["antenv.axon_hooks"] = mod
        antenv.axon_hooks = mod
        from trn_agent_boot.trn_boot import _ntff_profile_via_ctypes  # noqa: PLC0415

        mod.set_axon_ntff_profile_hook(
            _ntff_profile_via_ctypes("/opt/axon/libaxon_pjrt.so")
        )
    except Exception:
        pass


def _build(qkv_bias: bool, proj_bias: bool):
    nc = bacc.Bacc("TRN2", target_bir_lowering=False, debug=False, num_devices=CORES)

    xT_d = nc.declare_dram_parameter("xT", [NCH, 128, T], F32R, isOutput=False)
    wq_d = nc.declare_dram_parameter("wq", [NCH, 128, HPG * D], F32R, isOutput=False)
    wk_d = nc.declare_dram_parameter("wk", [NCH, 128, HPG * D], F32R, isOutput=False)
    wv_d = nc.declare_dram_parameter("wv", [NCH, 128, HPG * D], F32R, isOutput=False)
    memT_d = nc.declare_dram_parameter("memT", [HPG, 128, MEM], F32R, isOutput=False)
    memV_d = nc.declare_dram_parameter("memV", [NMC, 128, HPG * D], F32R, isOutput=False)
    # Wp regrouped by n-block: [nb][128(d of head h at cols h*512.. wait: layout below]
    # wp[nb] : [4*128(d rows as 4 chunks stacked on partitions? no..)]
    # Layout: wp_d[h] = [128(d), C] rows of W_proj for local head h.
    wp_d = nc.declare_dram_parameter("wp", [HPG, 128, C], F32R, isOutput=False)
    masks_d = nc.declare_dram_parameter("masks", [4, 128, TB], F32, isOutput=False)
    onesc_d = nc.declare_dram_parameter("onesc", [128, 1], F32R, isOutput=False)
    ones512_d = nc.declare_dram_parameter("ones512", [1, TB], F32R, isOutput=False)
    if qkv_bias:
        bq_d = nc.declare_dram_parameter("bq", [1, HPG * D], F32R, isOutput=False)
        bk_d = nc.declare_dram_parameter("bk", [1, HPG * D], F32R, isOutput=False)
        bv_d = nc.declare_dram_parameter("bv", [1, HPG * D], F32R, isOutput=False)
    if proj_bias:
        bp_d = nc.declare_dram_parameter("bp", [1, C], F32R, isOutput=False)
    y_d = nc.declare_dram_parameter("y", [T, C], F32, isOutput=True)

    with tile.TileContext(nc) as tc, nc.allow_low_precision(
        reason="float32r matmul inputs"
    ), contextlib.ExitStack() as ctx:
        prod = ctx.enter_context(tc.tile_pool(name="prod", bufs=1))
        const = ctx.enter_context(tc.tile_pool(name="const", bufs=1))

        # Persistent products (fp32r so they can feed matmuls directly).
        qT = [prod.tile([128, T], F32R, tag=f"qT{h}", name=f"qT{h}") for h in range(HPG)]
        kT = [prod.tile([128, T], F32R, tag=f"kT{h}", name=f"kT{h}") for h in range(HPG)]
        v_all = prod.tile([128, NCH * HPG * D], F32R)  # [t-chunk-major, 512 cols each]

        ones_col = const.tile([128, 1], F32R)
        nc.sync.dma_start(out=ones_col, in_=onesc_d[:])
        ones512 = const.tile([1, TB], F32R)
        nc.sync.dma_start(out=ones512, in_=ones512_d[:])
        if qkv_bias:
            bq_t = const.tile([1, HPG * D], F32R, tag="bq")
            bk_t = const.tile([1, HPG * D], F32R, tag="bk")
            bv_t = const.tile([1, HPG * D], F32R, tag="bv")
            nc.sync.dma_start(out=bq_t, in_=bq_d[:])
            nc.sync.dma_start(out=bk_t, in_=bk_d[:])
            nc.sync.dma_start(out=bv_t, in_=bv_d[:])
        if proj_bias:
            bp_t = const.tile([1, C], F32R)
            nc.sync.dma_start(out=bp_t, in_=bp_d[:])

        # ---------------- Phase 1: QKV projection ----------------
        # Three passes (q, k, v); each streams x chunks and runs 4 parallel
        # 16-matmul PSUM chains (bufs=2 per tag -> 8 banks, full rotation).
        with contextlib.ExitStack() as p1:
            wpool = p1.enter_context(tc.tile_pool(name="wpool", bufs=2))
            xpool = p1.enter_context(tc.tile_pool(name="xpool", bufs=4))
            psq = p1.enter_context(tc.tile_pool(name="psq", bufs=2, space="PSUM"))

            def proj_pass(w_dram, bias_tile, dests):
                # dests(h, j) -> SBUF AP [128, TB] for the (head, t-block) result
                wt = wpool.tile([128, NCH * HPG * D], F32R, tag="w")
                for i in range(NCH):
                    nc.sync.dma_start(out=wt[:, i * 512 : (i + 1) * 512], in_=w_dram[i])
                for j in range(NTB):
                    ps = [psq.tile([128, TB], F32, tag=f"c{h}", name=f"psc{h}") for h in range(HPG)]
                    for i in range(NCH):
                        xt = xpool.tile([128, TB], F32R, tag="x")
                        nc.sync.dma_start(
                            out=xt, in_=xT_d[i, :, j * TB : (j + 1) * TB]
                        )
                        for h in range(HPG):
                            nc.tensor.matmul(
                                ps[h],
                                wt[:, i * 512 + h * D : i * 512 + (h + 1) * D],
                                xt,
                                start=(i == 0),
                                stop=(i == NCH - 1 and bias_tile is None),
                            )
                    if bias_tile is not None:
                        for h in range(HPG):
                            nc.tensor.matmul(
                                ps[h],
                                bias_tile[0:1, h * D : (h + 1) * D],
                                ones512,
                                start=False,
                                stop=True,
                            )
                    for h in range(HPG):
                        nc.vector.tensor_copy(out=dests(h, j), in_=ps[h])

            proj_pass(wq_d, bq_t if qkv_bias else None,
                      lambda h, j: qT[h][:, j * TB : (j + 1) * TB])
            proj_pass(wk_d, bk_t if qkv_bias else None,
                      lambda h, j: kT[h][:, j * TB : (j + 1) * TB])

            # v pass: out v[t,d] per 128-row t-subtile; lhsT = x chunk slice.
            wt = wpool.tile([128, NCH * HPG * D], F32R, tag="w")
            for i in range(NCH):
                nc.sync.dma_start(out=wt[:, i * 512 : (i + 1) * 512], in_=wv_d[i])
            for j in range(NTB):
                ps = [psq.tile([128, TB], F32, tag=f"c{m}", name=f"psv{m}") for m in range(4)]
                for i in range(NCH):
                    xt = xpool.tile([128, TB], F32R, tag="x")
                    nc.sync.dma_start(out=xt, in_=xT_d[i, :, j * TB : (j + 1) * TB])
                    for m in range(4):
                        nc.tensor.matmul(
                            ps[m],
                            xt[:, m * 128 : (m + 1) * 128],
                            wt[:, i * 512 : (i + 1) * 512],
                            start=(i == 0),
                            stop=(i == NCH - 1 and not qkv_bias),
                        )
                if qkv_bias:
                    for m in range(4):
                        nc.tensor.matmul(
                            ps[m], ones512[0:1, 0:128], bv_t, start=False, stop=True
                        )
                for m in range(4):
                    c = j * 4 + m
                    nc.vector.tensor_copy(
                        out=v_all[:, c * 512 : (c + 1) * 512], in_=ps[m]
                    )

        # ---------------- Phase 2+3: attention + projection ----------------
        with contextlib.ExitStack() as p2:
            big = p2.enter_context(tc.tile_pool(name="big", bufs=1))
            wppool = p2.enter_context(tc.tile_pool(name="wppool", bufs=2))
            work = p2.enter_context(tc.tile_pool(name="work", bufs=3))
            dwork = p2.enter_context(tc.tile_pool(name="dwork", bufs=2))
            otn_pool = p2.enter_context(tc.tile_pool(name="otn", bufs=2))
            ps_sc = p2.enter_context(tc.tile_pool(name="ps_sc", bufs=2, space="PSUM"))
            ps_ot = p2.enter_context(tc.tile_pool(name="ps_ot", bufs=2, space="PSUM"))
            ps_sm = p2.enter_context(tc.tile_pool(name="ps_sm", bufs=1, space="PSUM"))
            ps_rb = p2.enter_context(tc.tile_pool(name="ps_rb", bufs=1, space="PSUM"))
            ps_y = p2.enter_context(tc.tile_pool(name="ps_y", bufs=2, space="PSUM"))

            memT = big.tile([128, HPG * MEM], F32R, tag="memT")
            for h in range(HPG):
                nc.sync.dma_start(out=memT[:, h * MEM : (h + 1) * MEM], in_=memT_d[h])
            memV = big.tile([128, NMC * HPG * D], F32R, tag="memV")
            for cm in range(NMC):
                nc.sync.dma_start(out=memV[:, cm * 512 : (cm + 1) * 512], in_=memV_d[cm])
            masks = big.tile([128, 4 * TB], F32, tag="masks")
            for k in range(4):
                nc.sync.dma_start(out=masks[:, k * TB : (k + 1) * TB], in_=masks_d[k])

            for j in range(NTB):
                otn = [otn_pool.tile([128, TB], F32R, tag=f"h{h}", name=f"otn{h}") for h in range(HPG)]
                for h in range(HPG):
                    q_j = qT[h][:, j * TB : (j + 1) * TB]
                    # chunk list: (lhsT-KT, lhsT-V, mask_idx or None)
                    chunks = []
                    for c in range(4 * j + 4):
                        ktile = kT[h][:, c * 128 : (c + 1) * 128]
                        vtile = v_all[:, c * 512 + h * D : c * 512 + (h + 1) * D]
                        kmask = c - 4 * j if c >= 4 * j else None
                        chunks.append((ktile, vtile, kmask))
                    for cm in range(NMC):
                        ktile = memT[:, h * MEM + cm * 128 : h * MEM + (cm + 1) * 128]
                        vtile = memV[:, cm * 512 + h * D : cm * 512 + (h + 1) * D]
                        chunks.append((ktile, vtile, None))

                    ot = ps_ot.tile([128, TB], F32, tag="ot")
                    denomP = dwork.tile([128, TB], F32R, tag="denomP")
                    last = len(chunks) - 1
                    for idx, (ktile, vtile, kmask) in enumerate(chunks):
                        sc = ps_sc.tile([128, TB], F32, tag="sc")
                        nc.tensor.matmul(sc, ktile, q_j, start=True, stop=True)
                        if kmask is not None:
                            nc.vector.tensor_add(
                                out=sc, in0=sc,
                                in1=masks[:, kmask * TB : (kmask + 1) * TB],
                            )
                        e = work.tile([128, TB], F32R, tag="e")
                        nc.scalar.activation(out=e, in_=sc, func=EXP, scale=SCALE)
                        if idx == 0:
                            nc.vector.tensor_copy(out=denomP, in_=e)
                        else:
                            nc.vector.tensor_add(out=denomP, in0=denomP, in1=e)
                        nc.tensor.matmul(
                            ot, vtile, e, start=(idx == 0), stop=(idx == last)
                        )
                    den = ps_sm.tile([1, TB], F32, tag="den")
                    nc.tensor.matmul(den, ones_col, denomP, start=True, stop=True)
                    recip = dwork.tile([1, TB], F32R, tag="recip")
                    nc.vector.reciprocal(out=recip, in_=den)
                    rbp = ps_rb.tile([128, TB], F32, tag="rb")
                    nc.tensor.matmul(rbp, ones512[0:1, 0:128], recip, start=True, stop=True)
                    rb = dwork.tile([128, TB], F32R, tag="rbs")
                    nc.vector.tensor_copy(out=rb, in_=rbp)
                    nc.vector.tensor_mul(out=otn[h], in0=ot, in1=rb)

                # Output projection for this t-block.
                for nb in range(C // TB):
                    wpt = wppool.tile([128, HPG * TB], F32R, tag="wp")
                    for h in range(HPG):
                        nc.sync.dma_start(
                            out=wpt[:, h * TB : (h + 1) * TB],
                            in_=wp_d[h, :, nb * TB : (nb + 1) * TB],
                        )
                    for m in range(4):
                        py = ps_y.tile([128, TB], F32, tag="y")
                        for h in range(HPG):
                            nc.tensor.matmul(
                                py,
                                otn[h][:, m * 128 : (m + 1) * 128],
                                wpt[:, h * TB : (h + 1) * TB],
                                start=(h == 0),
                                stop=(h == HPG - 1 and not proj_bias),
                            )
                        if proj_bias:
                            nc.tensor.matmul(
                                py, ones512[0:1, 0:128],
                                bp_t[0:1, nb * TB : (nb + 1) * TB],
                                start=False, stop=True,
                            )
                        yt = work.tile([128, TB], F32, tag="yt")
                        nc.vector.tensor_copy(out=yt, in_=py)
                        nc.sync.dma_start(
                            out=y_d[j * TB + m * 128 : j * TB + (m + 1) * 128,
                                    nb * TB : (nb + 1) * TB],
                            in_=yt,
                        )

    nc.compile()
    return nc


def _prep_core_inputs(c, x, W_qkv, b_qkv, memory_bank, W_proj, b_proj,
                      masks, qkv_bias, proj_bias):
    b, hg = c // HG, c % HG
    cols = slice(512 * hg, 512 * hg + 512)
    ca = np.ascontiguousarray
    xT = ca(x[b].T.reshape(NCH, 128, T))
    m = {
        "xT": xT,
        "wq": ca(W_qkv[:, cols].reshape(NCH, 128, HPG * D)),
        "wk": ca(W_qkv[:, C:][:, cols].reshape(NCH, 128, HPG * D)),
        "wv": ca(W_qkv[:, 2 * C:][:, cols].reshape(NCH, 128, HPG * D)),
        "memT": ca(np.stack([
            memory_bank[0][:, 512 * hg + 128 * h : 512 * hg + 128 * (h + 1)].T
            for h in range(HPG)])),
        "memV": ca(memory_bank[0][:, cols].reshape(NMC, 128, HPG * D)),
        "wp": ca(W_proj[512 * hg : 512 * (hg + 1), :].reshape(HPG, 128, C)),
        "masks": masks,
        "onesc": np.ones((128, 1), np.float32),
        "ones512": np.ones((1, TB), np.float32),
    }
    if qkv_bias:
        m["bq"] = ca(b_qkv[cols].reshape(1, HPG * D))
        m["bk"] = ca(b_qkv[C:][cols].reshape(1, HPG * D))
        m["bv"] = ca(b_qkv[2 * C:][cols].reshape(1, HPG * D))
    if proj_bias:
        m["bp"] = ca((b_proj / HG).reshape(1, C).astype(np.float32))
    return m


def kernel(x, W_qkv, b_qkv, memory_bank, W_proj, b_proj):
    global LAST_EXEC_TIME_NS
    _install_ntff_hook()
    x = np.asarray(x, np.float32)
    W_qkv = np.asarray(W_qkv, np.float32)
    b_qkv = np.asarray(b_qkv, np.float32)
    memory_bank = np.asarray(memory_bank, np.float32)
    W_proj = np.asarray(W_proj, np.float32)
    b_proj = np.asarray(b_proj, np.float32)

    qkv_bias = bool(np.any(b_qkv != 0))
    proj_bias = bool(np.any(b_proj != 0))

    key = (qkv_bias, proj_bias)
    if key not in _CACHE:
        _CACHE[key] = _build(qkv_bias, proj_bias)
    nc = _CACHE[key]

    # mask tile k: rows s = t0 + 128k + i, cols t = t0 + jj; allowed iff s <= t
    i_idx = np.arange(128)[:, None]
    jj = np.arange(TB)[None, :]
    masks = np.stack([
        np.where(i_idx + 128 * k <= jj, 0.0, NEGMASK).astype(np.float32)
        for k in range(4)
    ])

    in_maps = [
        _prep_core_inputs(c, x, W_qkv, b_qkv, memory_bank, W_proj, b_proj,
                          masks, qkv_bias, proj_bias)
        for c in range(CORES)
    ]
    trace = os.environ.get("KERNEL_TRACE", "0") == "1"
    res = run_bass_kernel_spmd(nc, in_maps, list(range(CORES)), trace=trace)
    LAST_EXEC_TIME_NS = res.exec_time_ns

    out = np.empty((B, T, C), np.float32)
    for b in range(B):
        acc = res.results[b * HG][ "y"].astype(np.float32)
        for g in range(1, HG):
            acc = acc + res.results[b * HG + g]["y"]
        out[b] = acc
    return out


# revision 4
# speedup vs baseline: 1.1147x; 1.1147x over previous
"""MetaAttention Trainium2 kernel.

Problem: B=2, T=2048, C=2048, H=16 heads (D=128), 1024-token memory bank
appended (unprojected) to K/V, causal mask over real tokens, memory tokens
attendable by every query; QKV projection + output projection.

Sharding: 8 cores = 2 batch groups x 4 head groups (4 heads each).
Per core: QKV projection for its 4 heads' columns over its batch,
attention for 4 (head) x 1 (batch), partial output projection with its
head-rows of W_proj. Host sums the 4 partials per batch.

All matmuls run in float32r (full-rate fp32 path, ~1e-4 accurate).
Layouts are chosen so no on-device transposes are needed anywhere:
  scoresT[s,t] = KT_chunk.T @ qT      (lhsT = KT chunk [d,s])
  OT[d,t]     += V_chunk.T @ expT     (lhsT = V chunk [s,d], rhs = expT[s,t])
  y[t,n]      += OT_chunk.T @ Wp      (lhsT = OT chunk [d,t])
Causal masking multiplies exp tiles by 0/1 masks (GpSimd) for the 4
diagonal s-chunks of each t-block; fully-masked chunks are skipped.
Softmax denominator: deferred DVE accumulation of exp tiles (two
accumulators), then a ones-matmul reduces partitions, reciprocal,
broadcast back via a second ones-matmul. Max-subtraction is skipped
(scores are O(5); exp is safe in fp32).
Attention processes two heads in interleaved lockstep so the tensor
engine always has an independent matmul available (keeps HAM warm).
"""

import contextlib
import os
import sys
import types

import numpy as np

for _p in ("/opt/trn_rl_repo", "/root/.axon_site/_ro/trn_rl_repo"):
    if os.path.isdir(_p) and _p not in sys.path:
        sys.path.append(_p)

import concourse.bass as bass  # noqa: E402
import concourse.mybir as mybir  # noqa: E402
import concourse.tile as tile  # noqa: E402
from concourse import bacc  # noqa: E402
from concourse.bass_utils import run_bass_kernel_spmd  # noqa: E402

F32 = mybir.dt.float32
F32R = mybir.dt.float32r
EXP = mybir.ActivationFunctionType.Exp
COPY = mybir.ActivationFunctionType.Copy

B, T, C = 2, 2048, 2048
H, D, MEM = 16, 128, 1024
HPG = 4            # heads per group (per core)
HG = H // HPG      # head groups
CORES = 8
TB = 512           # t-block width
NTB = T // TB      # 4 t-blocks
NCH = C // 128     # 16 contraction chunks
NMC = MEM // 128   # 8 memory s-chunks
SCALE = 1.0 / float(np.sqrt(D))

LAST_EXEC_TIME_NS = None
_CACHE = {}


def _install_ntff_hook():
    """Register the axon NTFF profiling hook if the image lacks antenv.axon_hooks."""
    try:
        import antenv  # noqa: PLC0415

        if "antenv.axon_hooks" in sys.modules:
            return
        mod = types.ModuleType("antenv.axon_hooks")
        state = {"hook": None}
        mod.set_axon_ntff_profile_hook = lambda h: state.__setitem__("hook", h)
        mod.get_axon_ntff_profile_hook = lambda: state["hook"]
        sys.modules["antenv.axon_hooks"] = mod
        antenv.axon_hooks = mod
        from trn_agent_boot.trn_boot import _ntff_profile_via_ctypes  # noqa: PLC0415

        mod.set_axon_ntff_profile_hook(
            _ntff_profile_via_ctypes("/opt/axon/libaxon_pjrt.so")
        )
    except Exception:
        pass


def _build(qkv_bias: bool, proj_bias: bool):
    nc = bacc.Bacc("TRN2", target_bir_lowering=False, debug=False, num_devices=CORES)

    xT_d = nc.declare_dram_parameter("xT", [NCH, 128, T], F32R, isOutput=False)
    wq_d = nc.declare_dram_parameter("wq", [NCH, 128, HPG * D], F32R, isOutput=False)
    wk_d = nc.declare_dram_parameter("wk", [NCH, 128, HPG * D], F32R, isOutput=False)
    wv_d = nc.declare_dram_parameter("wv", [NCH, 128, HPG * D], F32R, isOutput=False)
    memT_d = nc.declare_dram_parameter("memT", [HPG, 128, MEM], F32R, isOutput=False)
    memV_d = nc.declare_dram_parameter("memV", [NMC, 128, HPG * D], F32R, isOutput=False)
    wp_d = nc.declare_dram_parameter("wp", [HPG, 128, C], F32R, isOutput=False)
    masks_d = nc.declare_dram_parameter("masks", [4, 128, TB], F32, isOutput=False)
    onesc_d = nc.declare_dram_parameter("onesc", [128, 1], F32R, isOutput=False)
    ones512_d = nc.declare_dram_parameter("ones512", [1, TB], F32R, isOutput=False)
    if qkv_bias:
        bq_d = nc.declare_dram_parameter("bq", [1, HPG * D], F32R, isOutput=False)
        bk_d = nc.declare_dram_parameter("bk", [1, HPG * D], F32R, isOutput=False)
        bv_d = nc.declare_dram_parameter("bv", [1, HPG * D], F32R, isOutput=False)
    if proj_bias:
        bp_d = nc.declare_dram_parameter("bp", [1, C], F32R, isOutput=False)
    y_d = nc.declare_dram_parameter("y", [T, C], F32, isOutput=True)

    # DRAM bounce for qT (keeps SBUF under budget; reloaded per t-block).
    qs_d = nc.dram_tensor("qs", [NTB, 128, HPG * TB], F32R)

    with tile.TileContext(nc) as tc, nc.allow_low_precision(
        reason="float32r matmul inputs"
    ), contextlib.ExitStack() as ctx:
        prod = ctx.enter_context(tc.tile_pool(name="prod", bufs=1))
        const = ctx.enter_context(tc.tile_pool(name="const", bufs=1))

        kT = [prod.tile([128, T], F32R, tag=f"kT{h}", name=f"kT{h}") for h in range(HPG)]
        v_all = prod.tile([128, NCH * HPG * D], F32R)

        ones_col = const.tile([128, 1], F32R)
        nc.sync.dma_start(out=ones_col, in_=onesc_d[:])
        ones512 = const.tile([1, TB], F32R)
        nc.sync.dma_start(out=ones512, in_=ones512_d[:])
        if qkv_bias:
            bq_t = const.tile([1, HPG * D], F32R, tag="bq")
            bk_t = const.tile([1, HPG * D], F32R, tag="bk")
            bv_t = const.tile([1, HPG * D], F32R, tag="bv")
            nc.sync.dma_start(out=bq_t, in_=bq_d[:])
            nc.sync.dma_start(out=bk_t, in_=bk_d[:])
            nc.sync.dma_start(out=bv_t, in_=bv_d[:])
        if proj_bias:
            bp_t = const.tile([1, C], F32R)
            nc.sync.dma_start(out=bp_t, in_=bp_d[:])

        # ---------------- Phase 1: QKV projection ----------------
        # Three passes (q, k, v); each streams x in 4-chunk batches and runs
        # 4 parallel 16-matmul PSUM chains (4 tags x bufs=2 -> 8 banks).
        with contextlib.ExitStack() as p1:
            wpool = p1.enter_context(tc.tile_pool(name="wpool", bufs=2))
            xpool = p1.enter_context(tc.tile_pool(name="xpool", bufs=3))
            stg = p1.enter_context(tc.tile_pool(name="stg", bufs=3))
            psq = p1.enter_context(tc.tile_pool(name="psq", bufs=2, space="PSUM"))

            def load_w(w_dram):
                wt = wpool.tile([128, NCH * HPG * D], F32R, tag="w", name="wt")
                nc.sync.dma_start(
                    out=wt.rearrange("p (i n) -> p i n", i=NCH),
                    in_=w_dram.rearrange("i p n -> p i n"),
                )
                return wt

            def qk_pass(w_dram, bias_tile, is_q):
                wt = load_w(w_dram)
                for j in range(NTB):
                    ps = [psq.tile([128, TB], F32, tag=f"c{h}", name=f"ps{h}")
                          for h in range(HPG)]
                    for ib in range(NCH // 4):
                        xt = xpool.tile([128, 4 * TB], F32R, tag="x", name="xt")
                        nc.sync.dma_start(
                            out=xt.rearrange("p (i n) -> p i n", i=4),
                            in_=xT_d[4 * ib : 4 * ib + 4, :, j * TB : (j + 1) * TB]
                            .rearrange("i p n -> p i n"),
                        )
                        for ii in range(4):
                            i = 4 * ib + ii
                            for h in range(HPG):
                                nc.tensor.matmul(
                                    ps[h],
                                    wt[:, i * 512 + h * D : i * 512 + (h + 1) * D],
                                    xt[:, ii * TB : (ii + 1) * TB],
                                    start=(i == 0),
                                    stop=(i == NCH - 1 and bias_tile is None),
                                )
                    if bias_tile is not None:
                        for h in range(HPG):
                            nc.tensor.matmul(
                                ps[h], bias_tile[0:1, h * D : (h + 1) * D],
                                ones512, start=False, stop=True,
                            )
                    for h in range(HPG):
                        if is_q:
                            st = stg.tile([128, TB], F32R, tag="st", name="st")
                            nc.scalar.activation(out=st, in_=ps[h], func=COPY, scale=1.0)
                            nc.sync.dma_start(
                                out=qs_d[j, :, h * TB : (h + 1) * TB], in_=st
                            )
                        else:
                            nc.scalar.activation(
                                out=kT[h][:, j * TB : (j + 1) * TB],
                                in_=ps[h], func=COPY, scale=1.0,
                            )

            qk_pass(wq_d, bq_t if qkv_bias else None, is_q=True)
            qk_pass(wk_d, bk_t if qkv_bias else None, is_q=False)

            # v pass: out v[t,d] per 128-row t-subtile; lhsT = x chunk slice.
            wt = load_w(wv_d)
            for j in range(NTB):
                ps = [psq.tile([128, TB], F32, tag=f"c{m}", name=f"psv{m}")
                      for m in range(4)]
                for ib in range(NCH // 4):
                    xt = xpool.tile([128, 4 * TB], F32R, tag="x", name="xtv")
                    nc.sync.dma_start(
                        out=xt.rearrange("p (i n) -> p i n", i=4),
                        in_=xT_d[4 * ib : 4 * ib + 4, :, j * TB : (j + 1) * TB]
                        .rearrange("i p n -> p i n"),
                    )
                    for ii in range(4):
                        i = 4 * ib + ii
                        for m in range(4):
                            nc.tensor.matmul(
                                ps[m],
                                xt[:, ii * TB + m * 128 : ii * TB + (m + 1) * 128],
                                wt[:, i * 512 : (i + 1) * 512],
                                start=(i == 0),
                                stop=(i == NCH - 1 and not qkv_bias),
                            )
                if qkv_bias:
                    for m in range(4):
                        nc.tensor.matmul(
                            ps[m], ones512[0:1, 0:128], bv_t, start=False, stop=True
                        )
                for m in range(4):
                    c = j * 4 + m
                    nc.scalar.activation(
                        out=v_all[:, c * 512 : (c + 1) * 512],
                        in_=ps[m], func=COPY, scale=1.0,
                    )

        # ---------------- Phase 2+3: attention + projection ----------------
        with contextlib.ExitStack() as p2:
            big = p2.enter_context(tc.tile_pool(name="big", bufs=1))
            qpool = p2.enter_context(tc.tile_pool(name="qpool", bufs=2))
            wppool = p2.enter_context(tc.tile_pool(name="wppool", bufs=2))
            epool = p2.enter_context(tc.tile_pool(name="epool", bufs=4))
            dpool = p2.enter_context(tc.tile_pool(name="dpool", bufs=2))
            otn_pool = p2.enter_context(tc.tile_pool(name="otnp", bufs=1))
            ypool = p2.enter_context(tc.tile_pool(name="ypool", bufs=3))
            ps_sc = p2.enter_context(tc.tile_pool(name="ps_sc", bufs=3, space="PSUM"))
            ps_ot = p2.enter_context(tc.tile_pool(name="ps_ot", bufs=1, space="PSUM"))
            ps_sm = p2.enter_context(tc.tile_pool(name="ps_sm", bufs=1, space="PSUM"))
            ps_y = p2.enter_context(tc.tile_pool(name="ps_y", bufs=2, space="PSUM"))

            memT = big.tile([128, HPG * MEM], F32R, tag="memT")
            nc.sync.dma_start(
                out=memT.rearrange("p (h n) -> p h n", h=HPG),
                in_=memT_d.rearrange("h p n -> p h n"),
            )
            memV = big.tile([128, NMC * HPG * D], F32R, tag="memV")
            nc.sync.dma_start(
                out=memV.rearrange("p (c n) -> p c n", c=NMC),
                in_=memV_d.rearrange("c p n -> p c n"),
            )
            masks = big.tile([128, 4 * TB], F32, tag="masks")
            nc.sync.dma_start(
                out=masks.rearrange("p (k n) -> p k n", k=4),
                in_=masks_d.rearrange("k p n -> p k n"),
            )

            for j in range(NTB):
                qj = qpool.tile([128, HPG * TB], F32R, tag="qj", name="qj")
                nc.sync.dma_start(out=qj, in_=qs_d[j])
                otn = [otn_pool.tile([128, TB], F32R, tag=f"h{h}", name=f"otn{h}")
                       for h in range(HPG)]

                for pair in ((0, 1), (2, 3)):
                    # chunk schedule: diagonal (masked) first, then full real,
                    # then memory chunks.
                    sched = []
                    for c in range(4 * j, 4 * j + 4):
                        sched.append(("diag", c))
                    for c in range(4 * j):
                        sched.append(("real", c))
                    for cm in range(NMC):
                        sched.append(("mem", cm))
                    last = len(sched) - 1

                    ot, den = {}, {}
                    for lane, h in enumerate(pair):
                        ot[h] = ps_ot.tile([128, TB], F32, tag=f"ot{lane}",
                                           name=f"ot{lane}")
                        den[h] = [
                            dpool.tile([128, TB], F32R, tag=f"dn{lane}{p}",
                                       name=f"dn{lane}{p}")
                            for p in range(2)
                        ]
                    for idx, (kind, c) in enumerate(sched):
                        for lane, h in enumerate(pair):
                            if kind == "mem":
                                ktile = memT[:, h * MEM + c * 128 : h * MEM + (c + 1) * 128]
                                vtile = memV[:, c * 512 + h * D : c * 512 + (h + 1) * D]
                            else:
                                ktile = kT[h][:, c * 128 : (c + 1) * 128]
                                vtile = v_all[:, c * 512 + h * D : c * 512 + (h + 1) * D]
                            sc = ps_sc.tile([128, TB], F32, tag="sc", name="sc")
                            nc.tensor.matmul(
                                sc, ktile, qj[:, h * TB : (h + 1) * TB],
                                start=True, stop=True,
                            )
                            e = epool.tile([128, TB], F32R, tag=f"e{lane}",
                                           name=f"e{lane}")
                            nc.scalar.activation(out=e, in_=sc, func=EXP, scale=SCALE)
                            if kind == "diag":
                                k_idx = c - 4 * j
                                nc.gpsimd.tensor_mul(
                                    out=e, in0=e,
                                    in1=masks[:, k_idx * TB : (k_idx + 1) * TB],
                                )
                            acc = den[h][idx % 2]
                            if idx < 2:
                                nc.vector.tensor_copy(out=acc, in_=e)
                            else:
                                nc.vector.tensor_add(out=acc, in0=acc, in1=e)
                            nc.tensor.matmul(
                                ot[h], vtile, e, start=(idx == 0), stop=(idx == last)
                            )
                    for lane, h in enumerate(pair):
                        dsum = den[h][0]
                        nc.vector.tensor_add(out=dsum, in0=dsum, in1=den[h][1])
                        dn_ps = ps_sm.tile([1, TB], F32, tag="sm", name="dn_ps")
                        nc.tensor.matmul(dn_ps, ones_col, dsum, start=True, stop=True)
                        recip = dpool.tile([1, TB], F32R, tag=f"rc{lane}",
                                           name=f"recip{lane}")
                        nc.vector.reciprocal(out=recip, in_=dn_ps)
                        rb_ps = ps_sm.tile([128, TB], F32, tag="sm", name="rb_ps")
                        nc.tensor.matmul(rb_ps, ones512[0:1, 0:128], recip,
                                         start=True, stop=True)
                        rb = dpool.tile([128, TB], F32R, tag=f"rb{lane}",
                                        name=f"rb{lane}")
                        nc.vector.tensor_copy(out=rb, in_=rb_ps)
                        nc.vector.tensor_mul(out=otn[h], in0=ot[h], in1=rb)

                # Output projection for this t-block.
                for nb in range(C // TB):
                    wpt = wppool.tile([128, HPG * TB], F32R, tag="wp", name="wpt")
                    nc.sync.dma_start(
                        out=wpt.rearrange("p (h n) -> p h n", h=HPG),
                        in_=wp_d[:, :, nb * TB : (nb + 1) * TB]
                        .rearrange("h p n -> p h n"),
                    )
                    for m in range(4):
                        py = ps_y.tile([128, TB], F32, tag="y", name="py")
                        for h in range(HPG):
                            nc.tensor.matmul(
                                py,
                                otn[h][:, m * 128 : (m + 1) * 128],
                                wpt[:, h * TB : (h + 1) * TB],
                                start=(h == 0),
                                stop=(h == HPG - 1 and not proj_bias),
                            )
                        if proj_bias:
                            nc.tensor.matmul(
                                py, ones512[0:1, 0:128],
                                bp_t[0:1, nb * TB : (nb + 1) * TB],
                                start=False, stop=True,
                            )
                        yt = ypool.tile([128, TB], F32, tag="yt", name="yt")
                        nc.vector.tensor_copy(out=yt, in_=py)
                        nc.sync.dma_start(
                            out=y_d[j * TB + m * 128 : j * TB + (m + 1) * 128,
                                    nb * TB : (nb + 1) * TB],
                            in_=yt,
                        )

    nc.compile()
    return nc


def _prep_core_inputs(c, x, W_qkv, b_qkv, memory_bank, W_proj, b_proj,
                      masks, qkv_bias, proj_bias):
    b, hg = c // HG, c % HG
    cols = slice(512 * hg, 512 * hg + 512)
    ca = np.ascontiguousarray
    xT = ca(x[b].T.reshape(NCH, 128, T))
    m = {
        "xT": xT,
        "wq": ca(W_qkv[:, cols].reshape(NCH, 128, HPG * D)),
        "wk": ca(W_qkv[:, C:][:, cols].reshape(NCH, 128, HPG * D)),
        "wv": ca(W_qkv[:, 2 * C:][:, cols].reshape(NCH, 128, HPG * D)),
        "memT": ca(np.stack([
            memory_bank[0][:, 512 * hg + 128 * h : 512 * hg + 128 * (h + 1)].T
            for h in range(HPG)])),
        "memV": ca(memory_bank[0][:, cols].reshape(NMC, 128, HPG * D)),
        "wp": ca(W_proj[512 * hg : 512 * (hg + 1), :].reshape(HPG, 128, C)),
        "masks": masks,
        "onesc": np.ones((128, 1), np.float32),
        "ones512": np.ones((1, TB), np.float32),
    }
    if qkv_bias:
        m["bq"] = ca(b_qkv[cols].reshape(1, HPG * D))
        m["bk"] = ca(b_qkv[C:][cols].reshape(1, HPG * D))
        m["bv"] = ca(b_qkv[2 * C:][cols].reshape(1, HPG * D))
    if proj_bias:
        m["bp"] = ca((b_proj / HG).reshape(1, C).astype(np.float32))
    return m


def kernel(x, W_qkv, b_qkv, memory_bank, W_proj, b_proj):
    global LAST_EXEC_TIME_NS
    _install_ntff_hook()
    x = np.asarray(x, np.float32)
    W_qkv = np.asarray(W_qkv, np.float32)
    b_qkv = np.asarray(b_qkv, np.float32)
    memory_bank = np.asarray(memory_bank, np.float32)
    W_proj = np.asarray(W_proj, np.float32)
    b_proj = np.asarray(b_proj, np.float32)

    qkv_bias = bool(np.any(b_qkv != 0))
    proj_bias = bool(np.any(b_proj != 0))

    key = (qkv_bias, proj_bias)
    if key not in _CACHE:
        _CACHE[key] = _build(qkv_bias, proj_bias)
    nc = _CACHE[key]

    # 0/1 mask tile k: rows s = t0 + 128k + i, cols t = t0 + jj; allowed iff s <= t
    i_idx = np.arange(128)[:, None]
    jj = np.arange(TB)[None, :]
    masks = np.stack([
        (i_idx + 128 * k <= jj).astype(np.float32) for k in range(4)
    ])

    in_maps = [
        _prep_core_inputs(c, x, W_qkv, b_qkv, memory_bank, W_proj, b_proj,
                          masks, qkv_bias, proj_bias)
        for c in range(CORES)
    ]
    trace = os.environ.get("KERNEL_TRACE", "0") == "1"
    res = run_bass_kernel_spmd(nc, in_maps, list(range(CORES)), trace=trace)
    LAST_EXEC_TIME_NS = res.exec_time_ns

    out = np.empty((B, T, C), np.float32)
    for b in range(B):
        acc = res.results[b * HG]["y"].astype(np.float32)
        for g in range(1, HG):
            acc = acc + res.results[b * HG + g]["y"]
        out[b] = acc
    return out
